# revision 11
# baseline (speedup 1.0000x reference)
"""Causal single-head attention on 8 Trainium2 NeuronCores.

Problem: x[4, 2048, 1024] fp32, Wq/Wk/Wv[1024, 1024] fp32.
  q,k,v = x@Wq, x@Wk, x@Wv ; out = softmax(mask(q k^T)/32) @ v

Sharding (SPMD — one program, 8 cores, per-core data):
  core = 2*b + h  handles batch b, queries {t : t % 2 == h} (1024 queries).
  The interleaved (mod-2) query split makes the causal block structure
  shape-identical across cores: per-core q-block jb (256 queries, spanning
  global positions [512*jb, 512*jb+512)) needs k-tiles 0..4*(jb+1)-1 on
  every core.  Causal masking inside the 4 diagonal k-tiles depends only on
  (u = t-4*jb, parity h) => 4 mask tiles passed as per-core data.

Layouts:
  - host passes x[b]^T ([D, T]) so Q^T/K^T/V all come out of matmuls with
    no on-chip transposes (contraction over d_in needs d_in on partitions).
  - scores are computed transposed ([k, q]) so that expS can feed the
    attn @ V matmul directly as the stationary operand (lhsT = expS[k, q]).
  - softmax denominator = ones-matmul over the key partition dim, and no
    max-subtraction (logits/32 are ~N(0, 0.41^2); exp never overflows).

Dtypes: bf16 matmul inputs for projections and scores (fp32 PSUM accum),
fp32 V / expS with float32r (full-rate fp32) context matmuls, fp32 softmax.
"""

import os
import numpy as np
import ml_dtypes

import concourse.bass as bass
import concourse.mybir as mybir
import concourse.tile as tile
from concourse import bacc

F32 = mybir.dt.float32
F32R = mybir.dt.float32r
BF16 = mybir.dt.bfloat16
BF16_NP = ml_dtypes.bfloat16

B, T, D = 4, 2048, 1024
P = 128
DC = D // P          # 8 contraction chunks
NW = T // 512        # 4 token windows (K/V projection granularity)
NT = T // P          # 16 key tiles
QB = 256             # queries per q-block (per core)
NJB = (T // 2) // QB # 4 q-blocks per core
SCALE = 1.0 / 32.0   # 1/sqrt(D)

# V / expS storage dtype: fp32 + float32r matmuls (accurate, full rate at
# N>=256).  Flip to BF16 if hardware shows fp32r matmuls are slow.
V_F32 = True
V_DT = F32R if V_F32 else BF16   # fp32r: walrus requires producers to round
MASK_NEG = -1.0e9
# Split K/V projection across the core pair (each core projects its own 1024
# tokens) and AllGather the halves — halves the duplicated projection work.
KV_SPLIT = True
PAIRS = [[0, 1], [2, 3], [4, 5], [6, 7]]
_EXP = mybir.ActivationFunctionType.Exp


def _emit(nc, tc, xT_d, xTq_d, wq_d, wk_d, wv_d, masks_d, out_d):
    HT = T // 2  # queries per core

    def mm(out, lhsT, rhs, start, stop, **kw):
        if out.dtype == F32 and lhsT.dtype == F32:
            lhsT = lhsT.bitcast(F32R)
            rhs = rhs.bitcast(F32R)
        nc.tensor.matmul(out, lhsT, rhs, start=start, stop=stop, **kw)

    with (
        tc.sbuf_pool(name="persist", bufs=1) as persist,
        tc.sbuf_pool(name="recipp", bufs=2) as recip_pool,
        tc.sbuf_pool(name="outp", bufs=4) as out_pool,
        tc.psum_pool(name="p512", bufs=4) as p512,
        tc.psum_pool(name="p256", bufs=3) as p256,
        tc.psum_pool(name="pden", bufs=1) as pden,
    ):
        # ---- persistent SBUF tensors ----
        K_sb = persist.tile([P, DC * T], BF16, tag="K", name="K_sb")
        V_sb = persist.tile([P, NT * D], V_DT, tag="V", name="V_sb")
        Q_sb = persist.tile([P, DC * HT], BF16, tag="Q", name="Q_sb")
        expS = persist.tile([P, NT * QB], V_DT, tag="E", name="expS")
        mask_sb = persist.tile([P, 4 * QB], F32, tag="M", name="mask_sb")
        ones_sb = persist.tile([P, 1], V_DT, tag="O", name="ones_sb")
        ones_f32 = persist.tile([P, 1], F32, tag="O32", name="ones_f32")
        nc.vector.memset(ones_f32, 1.0)
        nc.scalar.copy(out=ones_sb, in_=ones_f32)
        for u in range(4):
            nc.sync.dma_start(out=mask_sb[:, u * QB:(u + 1) * QB], in_=masks_d[u])

        # ---- K^T and V projections ----
        # KV_SPLIT: each core projects only its own token half, bounces the
        # halves through DRAM, AllGathers within the core pair, then reads
        # the full K^T/V back into SBUF.
        NWO = NW // 2 if KV_SPLIT else NW  # token windows this core projects
        with (
            tc.sbuf_pool(name="wkvp", bufs=1) as wkv_pool,
            tc.sbuf_pool(name="xtwp", bufs=2) as xtw_pool,
            tc.sbuf_pool(name="stgp", bufs=6) as stg_pool,
            tc.tile_pool(name="drp", bufs=1, space="DRAM") as dr_pool,
            nc.named_scope("kv_proj"),
        ):
            if KV_SPLIT:
                kloc = dr_pool.tile([D, T // 2], BF16, tag="kloc", name="kloc")
                vloc = dr_pool.tile([T // 2, D], V_DT, tag="vloc", name="vloc")
                kg = dr_pool.tile([2, D, T // 2], BF16, tag="kg", name="kg")
                vg = dr_pool.tile([2, T // 2, D], V_DT, tag="vg", name="vg")
            wk_sb = wkv_pool.tile([P, DC * D], BF16, tag="wk", name="wk_sb")
            wv_sb = wkv_pool.tile([P, DC * D], BF16, tag="wv", name="wv_sb")
            for c in range(DC):
                nc.sync.dma_start(out=wk_sb[:, c * D:(c + 1) * D],
                                  in_=wk_d[c * P:(c + 1) * P, :])
                nc.sync.dma_start(out=wv_sb[:, c * D:(c + 1) * D],
                                  in_=wv_d[c * P:(c + 1) * P, :])
            for w in range(NWO):
                xtw = xtw_pool.tile([P, DC * 512], BF16, tag="xtw", name="xtw")
                for c in range(DC):
                    nc.sync.dma_start(
                        out=xtw[:, c * 512:(c + 1) * 512],
                        in_=xT_d[c * P:(c + 1) * P, 512 * w:512 * (w + 1)])
                # K^T[c2-chunk, window] = sum_c Wk[c, c2]^T @ xT[c, window]
                for c2 in range(DC):
                    ps = p512.tile([P, 512], F32, tag="mm512", name="ps_k")
                    for c in range(DC):
                        mm(ps, wk_sb[:, c * D + P * c2: c * D + P * (c2 + 1)],
                           xtw[:, c * 512:(c + 1) * 512], c == 0, c == DC - 1)
                    if KV_SPLIT:
                        st = stg_pool.tile([P, 512], BF16, tag="stk", name="stk")
                        nc.scalar.copy(out=st, in_=ps)
                        nc.sync.dma_start(
                            out=kloc[c2 * P:(c2 + 1) * P,
                                     512 * w:512 * (w + 1)],
                            in_=st)
                    else:
                        nc.scalar.copy(
                            out=K_sb[:, c2 * T + 512 * w: c2 * T + 512 * (w + 1)],
                            in_=ps)
                # V[tok-tile, d-half] = sum_c xT[c, tok]^T @ Wv[c, d-half]
                for ts in range(4):
                    t = 4 * w + ts
                    for n in range(2):
                        ps = p512.tile([P, 512], F32, tag="mm512", name="ps_v")
                        for c in range(DC):
                            mm(ps, xtw[:, c * 512 + P * ts: c * 512 + P * (ts + 1)],
                               wv_sb[:, c * D + 512 * n: c * D + 512 * (n + 1)],
                               c == 0, c == DC - 1)
                        if KV_SPLIT:
                            st = stg_pool.tile([P, 512], V_DT, tag="stv",
                                               name="stv")
                            nc.scalar.copy(out=st, in_=ps)
                            nc.sync.dma_start(
                                out=vloc[t * P:(t + 1) * P,
                                         512 * n:512 * (n + 1)],
                                in_=st)
                        else:
                            nc.scalar.copy(
                                out=V_sb[:, t * D + 512 * n: t * D + 512 * (n + 1)],
                                in_=ps)
            if KV_SPLIT:
                nc.gpsimd.collective_compute(
                    "AllGather", mybir.AluOpType.bypass, replica_groups=PAIRS,
                    ins=[kloc[:]], outs=[kg[:]])
                nc.gpsimd.collective_compute(
                    "AllGather", mybir.AluOpType.bypass, replica_groups=PAIRS,
                    ins=[vloc[:]], outs=[vg[:]])
                for c in range(DC):
                    for w in range(NW):
                        r, lw = divmod(w, 2)
                        nc.sync.dma_start(
                            out=K_sb[:, c * T + 512 * w: c * T + 512 * (w + 1)],
                            in_=kg[r, c * P:(c + 1) * P,
                                   512 * lw:512 * (lw + 1)])
                for t in range(NT):
                    r, lt = divmod(t, DC)
                    nc.sync.dma_start(
                        out=V_sb[:, t * D:(t + 1) * D],
                        in_=vg[r, lt * P:(lt + 1) * P, :])

        # ---- Q^T projection (own queries only), per q-block ----
        with (
            tc.sbuf_pool(name="wqp", bufs=1) as wq_pool,
            tc.sbuf_pool(name="xtqp", bufs=2) as xtq_pool,
            nc.named_scope("q_proj"),
        ):
            wq_sb = wq_pool.tile([P, DC * D], BF16, tag="wq", name="wq_sb")
            for c in range(DC):
                nc.sync.dma_start(out=wq_sb[:, c * D:(c + 1) * D],
                                  in_=wq_d[c * P:(c + 1) * P, :])
            for jp in range(NJB // 2):  # project two q-blocks at once (N=512)
                xtq = xtq_pool.tile([P, DC * 512], BF16, tag="xtq", name="xtq")
                for c in range(DC):
                    nc.sync.dma_start(
                        out=xtq[:, c * 512:(c + 1) * 512],
                        in_=xTq_d[c * P:(c + 1) * P, 512 * jp:512 * (jp + 1)])
                for c2 in range(DC):
                    ps = p512.tile([P, 512], F32, tag="mm512", name="ps_q")
                    for c in range(DC):
                        mm(ps, wq_sb[:, c * D + P * c2: c * D + P * (c2 + 1)],
                           xtq[:, c * 512:(c + 1) * 512], c == 0, c == DC - 1)
                    nc.scalar.copy(
                        out=Q_sb[:, c2 * HT + 512 * jp: c2 * HT + 512 * (jp + 1)],
                        in_=ps)

        # ---- attention, per q-block ----
        with nc.named_scope("attn"):
            for jb in range(NJB):
                kt = 4 * (jb + 1)  # k-tiles needed by this q-block
                # pass 1: scores^T -> exp (-> mask on the 4 diagonal tiles)
                for t in range(kt):
                    ps = p256.tile([P, QB], F32, tag="mm256", name="ps_s")
                    for c in range(DC):
                        mm(ps, K_sb[:, c * T + P * t: c * T + P * (t + 1)],
                           Q_sb[:, c * HT + QB * jb: c * HT + QB * (jb + 1)],
                           c == 0, c == DC - 1)
                    if t >= kt - 4:
                        u = t - (kt - 4)
                        nc.vector.tensor_add(ps, ps,
                                             mask_sb[:, u * QB:(u + 1) * QB])
                    nc.scalar.activation(out=expS[:, t * QB:(t + 1) * QB], in_=ps,
                                         func=_EXP, scale=SCALE)
                # denominators: den[q, s] = sum_k expS[k, q]
                den = pden.tile([P, 2], F32, tag="den", name="den")
                for s in range(2):
                    for t in range(kt):
                        # N=1 violates fp32r ISA restrictions -> plain fp32
                        nc.tensor.matmul(
                            den[:, s:s + 1],
                            expS[:, t * QB + P * s: t * QB + P * (s + 1)]
                            .bitcast(F32),
                            ones_sb.bitcast(F32), start=t == 0,
                            stop=t == kt - 1, skip_group_check=True)
                recip = recip_pool.tile([P, 2], F32, tag="recip", name="recip")
                nc.vector.reciprocal(recip, den)
                # pass 2: ctx[q, d] = sum_k expS[k, q] * V[k, d], then normalize
                for s in range(2):
                    for n in range(2):
                        ps = p512.tile([P, 512], F32, tag="mm512", name="ps_c")
                        for t in range(kt):
                            mm(ps, expS[:, t * QB + P * s: t * QB + P * (s + 1)],
                               V_sb[:, t * D + 512 * n: t * D + 512 * (n + 1)],
                               t == 0, t == kt - 1)
                        ot = out_pool.tile([P, 512], F32, tag="out", name="ot")
                        nc.vector.tensor_scalar_mul(ot, ps, recip[:, s:s + 1])
                        nc.sync.dma_start(
                            out=out_d[QB * jb + P * s: QB * jb + P * (s + 1),
                                      512 * n: 512 * (n + 1)],
                            in_=ot)


def build_nc():
    nc = bacc.Bacc("TRN2", target_bir_lowering=False, debug=False, num_devices=8)
    xT_d = nc.dram_tensor("xT", [D, T // 2 if KV_SPLIT else T], BF16,
                          kind="ExternalInput")
    xTq_d = nc.dram_tensor("xTq", [D, T // 2], BF16, kind="ExternalInput")
    wq_d = nc.dram_tensor("wq", [D, D], BF16, kind="ExternalInput")
    wk_d = nc.dram_tensor("wk", [D, D], BF16, kind="ExternalInput")
    wv_d = nc.dram_tensor("wv", [D, D], BF16, kind="ExternalInput")
    masks_d = nc.dram_tensor("masks", [4, P, QB], F32, kind="ExternalInput")
    out_d = nc.dram_tensor("out", [T // 2, D], F32, kind="ExternalOutput")
    with tile.TileContext(nc) as tc:
        _emit(nc, tc, xT_d[:], xTq_d[:], wq_d[:], wk_d[:], wv_d[:], masks_d[:],
              out_d[:])
    nc.compile()
    return nc


def make_masks(h):
    """Additive causal mask: 0 where key (128u + p) <= query (2j + h), else
    -1e9, within a 512-position diagonal window (positions relative to the
    q-block base).  Applied to raw scores before exp."""
    u = np.arange(4)[:, None, None]
    p = np.arange(P)[None, :, None]
    j = np.arange(QB)[None, None, :]
    vis = (128 * u + p <= 2 * j + h)
    return np.where(vis, 0.0, MASK_NEG).astype(np.float32)


def make_in_maps(x, W_query, W_key, W_value):
    wq = np.ascontiguousarray(W_query).astype(BF16_NP)
    wk = np.ascontiguousarray(W_key).astype(BF16_NP)
    wv = np.ascontiguousarray(W_value).astype(BF16_NP)
    masks = [make_masks(h) for h in range(2)]
    in_maps = []
    for core in range(8):
        b, h = divmod(core, 2)
        xb = np.asarray(x[b], dtype=np.float32)
        xkv = xb[1024 * h:1024 * (h + 1)] if KV_SPLIT else xb
        in_maps.append({
            "xT": np.ascontiguousarray(xkv.T).astype(BF16_NP),
            "xTq": np.ascontiguousarray(xb[h::2].T).astype(BF16_NP),
            "wq": wq, "wk": wk, "wv": wv,
            "masks": masks[h],
        })
    return in_maps


_NC_CACHE = {}
LAST_EXEC_NS = None


def kernel(x, W_query, W_key, W_value):
    global LAST_EXEC_NS
    from concourse.bass_utils import run_bass_kernel_spmd

    if "nc" not in _NC_CACHE:
        _NC_CACHE["nc"] = build_nc()
    nc = _NC_CACHE["nc"]

    in_maps = make_in_maps(x, W_query, W_key, W_value)
    trace = bool(os.environ.get("BASS_TRACE"))
    res = run_bass_kernel_spmd(nc, in_maps, core_ids=list(range(8)), trace=trace)
    LAST_EXEC_NS = res.exec_time_ns

    out = np.empty((B, T, D), dtype=np.float32)
    for core in range(8):
        b, h = divmod(core, 2)
        out[b, h::2, :] = res.results[core]["out"]
    return out


if __name__ == "__main__":
    import time
    t0 = time.time()
    nc = build_nc()
    print(f"build+compile took {time.time() - t0:.1f}s")
    n_inst = sum(len(getattr(e, 'instructions', [])) for e in nc.engines) \
        if hasattr(nc, 'engines') else -1
    print("built ok")


# revision 12
# speedup vs baseline: 1.2673x; 1.2673x over previous
"""Causal single-head attention on 8 Trainium2 NeuronCores.

Problem: x[4, 2048, 1024] fp32, Wq/Wk/Wv[1024, 1024] fp32.
  q,k,v = x@Wq, x@Wk, x@Wv ; out = softmax(mask(q k^T)/32) @ v

Sharding (SPMD — one program, 8 cores, per-core data):
  core = 2*b + h  handles batch b, queries {t : t % 2 == h} (1024 queries).
  The interleaved (mod-2) query split makes the causal block structure
  shape-identical across cores: per-core q-block jb (256 queries, spanning
  global positions [512*jb, 512*jb+512)) needs k-tiles 0..4*(jb+1)-1 on
  every core.  Causal masking inside the 4 diagonal k-tiles depends only on
  (u = t-4*jb, parity h) => 4 mask tiles passed as per-core data.

Layouts:
  - host passes x[b]^T ([D, T]) so Q^T/K^T/V all come out of matmuls with
    no on-chip transposes (contraction over d_in needs d_in on partitions).
  - scores are computed transposed ([k, q]) so that expS can feed the
    attn @ V matmul directly as the stationary operand (lhsT = expS[k, q]).
  - softmax denominator = ones-matmul over the key partition dim, and no
    max-subtraction (logits/32 are ~N(0, 0.41^2); exp never overflows).

Dtypes: bf16 matmul inputs for projections and scores (fp32 PSUM accum),
fp32 V / expS with float32r (full-rate fp32) context matmuls, fp32 softmax.
"""

import os
import numpy as np
import ml_dtypes

import concourse.bass as bass
import concourse.mybir as mybir
import concourse.tile as tile
from concourse import bacc

F32 = mybir.dt.float32
F32R = mybir.dt.float32r
BF16 = mybir.dt.bfloat16
BF16_NP = ml_dtypes.bfloat16

B, T, D = 4, 2048, 1024
P = 128
DC = D // P          # 8 contraction chunks
NW = T // 512        # 4 token windows (K/V projection granularity)
NT = T // P          # 16 key tiles
QB = 256             # queries per q-block (per core)
NJB = (T // 2) // QB # 4 q-blocks per core
SCALE = 1.0 / 32.0   # 1/sqrt(D)

# V / expS storage dtype: fp32 + float32r matmuls (accurate, full rate at
# N>=256).  Flip to BF16 if hardware shows fp32r matmuls are slow.
V_F32 = True
V_DT = F32R if V_F32 else BF16   # fp32r: walrus requires producers to round
MASK_NEG = -1.0e9
# Split the K^T projection across the core pair (each core projects its own
# 1024 tokens) and AllGather the halves; the 2 MB bf16 gather (~39 us) hides
# completely behind the full V + Q projections.  V stays locally projected —
# its 8 MB gather measured ~109 us and stalls the PE (tried, reverted).
K_SPLIT = True
PAIRS = [[0, 1], [2, 3], [4, 5], [6, 7]]
_EXP = mybir.ActivationFunctionType.Exp


def _emit(nc, tc, xT_d, xTk_d, xTq_d, wq_d, wk_d, wv_d, masks_d, out_d):
    HT = T // 2  # queries per core

    def mm(out, lhsT, rhs, start, stop, **kw):
        if out.dtype == F32 and lhsT.dtype == F32:
            lhsT = lhsT.bitcast(F32R)
            rhs = rhs.bitcast(F32R)
        nc.tensor.matmul(out, lhsT, rhs, start=start, stop=stop, **kw)

    with (
        tc.sbuf_pool(name="persist", bufs=1) as persist,
        tc.sbuf_pool(name="recipp", bufs=2) as recip_pool,
        tc.sbuf_pool(name="outp", bufs=4) as out_pool,
        tc.psum_pool(name="p512", bufs=4) as p512,
        tc.psum_pool(name="p256", bufs=3) as p256,
        tc.psum_pool(name="pden", bufs=1) as pden,
    ):
        # ---- persistent SBUF tensors ----
        K_sb = persist.tile([P, DC * T], BF16, tag="K", name="K_sb")
        V_sb = persist.tile([P, NT * D], V_DT, tag="V", name="V_sb")
        Q_sb = persist.tile([P, DC * HT], BF16, tag="Q", name="Q_sb")
        expS = persist.tile([P, NT * QB], V_DT, tag="E", name="expS")
        mask_sb = persist.tile([P, 4 * QB], F32, tag="M", name="mask_sb")
        ones_sb = persist.tile([P, 1], V_DT, tag="O", name="ones_sb")
        ones_f32 = persist.tile([P, 1], F32, tag="O32", name="ones_f32")
        nc.vector.memset(ones_f32, 1.0)
        nc.scalar.copy(out=ones_sb, in_=ones_f32)
        for u in range(4):
            nc.sync.dma_start(out=mask_sb[:, u * QB:(u + 1) * QB], in_=masks_d[u])

        # ---- projections: K^T (pair-split + AllGather) and V (local) ----
        with (
            tc.sbuf_pool(name="wkvp", bufs=1) as wkv_pool,
            tc.sbuf_pool(name="xtwp", bufs=2) as xtw_pool,
            tc.sbuf_pool(name="stgp", bufs=6) as stg_pool,
            tc.tile_pool(name="drp", bufs=1, space="DRAM") as dr_pool,
            nc.named_scope("kv_proj"),
        ):
            wk_sb = wkv_pool.tile([P, DC * D], BF16, tag="wk", name="wk_sb")
            wv_sb = wkv_pool.tile([P, DC * D], BF16, tag="wv", name="wv_sb")
            for c in range(DC):
                nc.sync.dma_start(out=wk_sb[:, c * D:(c + 1) * D],
                                  in_=wk_d[c * P:(c + 1) * P, :])
                nc.sync.dma_start(out=wv_sb[:, c * D:(c + 1) * D],
                                  in_=wv_d[c * P:(c + 1) * P, :])
            if K_SPLIT:
                kloc = dr_pool.tile([D, T // 2], BF16, tag="kloc", name="kloc")
                kg = dr_pool.tile([2, D, T // 2], BF16, tag="kg", name="kg")
                # K^T of own token half first, so the gather launches early
                for w in range(NW // 2):
                    xtk = xtw_pool.tile([P, DC * 512], BF16, tag="xtw",
                                        name="xtk")
                    for c in range(DC):
                        nc.sync.dma_start(
                            out=xtk[:, c * 512:(c + 1) * 512],
                            in_=xTk_d[c * P:(c + 1) * P, 512 * w:512 * (w + 1)])
                    for c2 in range(DC):
                        ps = p512.tile([P, 512], F32, tag="mm512", name="ps_k")
                        for c in range(DC):
                            mm(ps, wk_sb[:, c * D + P * c2: c * D + P * (c2 + 1)],
                               xtk[:, c * 512:(c + 1) * 512], c == 0, c == DC - 1)
                        st = stg_pool.tile([P, 512], BF16, tag="stk", name="stk")
                        nc.scalar.copy(out=st, in_=ps)
                        nc.sync.dma_start(
                            out=kloc[c2 * P:(c2 + 1) * P, 512 * w:512 * (w + 1)],
                            in_=st)
                nc.gpsimd.collective_compute(
                    "AllGather", mybir.AluOpType.bypass, replica_groups=PAIRS,
                    ins=[kloc[:]], outs=[kg[:]])
                for c in range(DC):
                    for w in range(NW):
                        r, lw = divmod(w, 2)
                        nc.sync.dma_start(
                            out=K_sb[:, c * T + 512 * w: c * T + 512 * (w + 1)],
                            in_=kg[r, c * P:(c + 1) * P, 512 * lw:512 * (lw + 1)])
            # V (full, local) — PE work here hides the K gather
            for w in range(NW):
                xtw = xtw_pool.tile([P, DC * 512], BF16, tag="xtw", name="xtw")
                for c in range(DC):
                    nc.sync.dma_start(
                        out=xtw[:, c * 512:(c + 1) * 512],
                        in_=xT_d[c * P:(c + 1) * P, 512 * w:512 * (w + 1)])
                if not K_SPLIT:
                    for c2 in range(DC):
                        ps = p512.tile([P, 512], F32, tag="mm512", name="ps_k")
                        for c in range(DC):
                            mm(ps, wk_sb[:, c * D + P * c2: c * D + P * (c2 + 1)],
                               xtw[:, c * 512:(c + 1) * 512], c == 0, c == DC - 1)
                        nc.scalar.copy(
                            out=K_sb[:, c2 * T + 512 * w: c2 * T + 512 * (w + 1)],
                            in_=ps)
                for ts in range(4):
                    t = 4 * w + ts
                    for n in range(2):
                        ps = p512.tile([P, 512], F32, tag="mm512", name="ps_v")
                        for c in range(DC):
                            mm(ps, xtw[:, c * 512 + P * ts: c * 512 + P * (ts + 1)],
                               wv_sb[:, c * D + 512 * n: c * D + 512 * (n + 1)],
                               c == 0, c == DC - 1)
                        nc.scalar.copy(
                            out=V_sb[:, t * D + 512 * n: t * D + 512 * (n + 1)],
                            in_=ps)

        # ---- Q^T projection (own queries only), per q-block ----
        with (
            tc.sbuf_pool(name="wqp", bufs=1) as wq_pool,
            tc.sbuf_pool(name="xtqp", bufs=2) as xtq_pool,
            nc.named_scope("q_proj"),
        ):
            wq_sb = wq_pool.tile([P, DC * D], BF16, tag="wq", name="wq_sb")
            for c in range(DC):
                nc.sync.dma_start(out=wq_sb[:, c * D:(c + 1) * D],
                                  in_=wq_d[c * P:(c + 1) * P, :])
            for jp in range(NJB // 2):  # project two q-blocks at once (N=512)
                xtq = xtq_pool.tile([P, DC * 512], BF16, tag="xtq", name="xtq")
                for c in range(DC):
                    nc.sync.dma_start(
                        out=xtq[:, c * 512:(c + 1) * 512],
                        in_=xTq_d[c * P:(c + 1) * P, 512 * jp:512 * (jp + 1)])
                for c2 in range(DC):
                    ps = p512.tile([P, 512], F32, tag="mm512", name="ps_q")
                    for c in range(DC):
                        mm(ps, wq_sb[:, c * D + P * c2: c * D + P * (c2 + 1)],
                           xtq[:, c * 512:(c + 1) * 512], c == 0, c == DC - 1)
                    nc.scalar.copy(
                        out=Q_sb[:, c2 * HT + 512 * jp: c2 * HT + 512 * (jp + 1)],
                        in_=ps)

        # ---- attention, per q-block ----
        with nc.named_scope("attn"):
            for jb in range(NJB):
                kt = 4 * (jb + 1)  # k-tiles needed by this q-block
                # pass 1: scores^T -> exp (-> mask on the 4 diagonal tiles)
                for t in range(kt):
                    ps = p256.tile([P, QB], F32, tag="mm256", name="ps_s")
                    for c in range(DC):
                        mm(ps, K_sb[:, c * T + P * t: c * T + P * (t + 1)],
                           Q_sb[:, c * HT + QB * jb: c * HT + QB * (jb + 1)],
                           c == 0, c == DC - 1)
                    if t >= kt - 4:
                        u = t - (kt - 4)
                        nc.vector.tensor_add(ps, ps,
                                             mask_sb[:, u * QB:(u + 1) * QB])
                    nc.scalar.activation(out=expS[:, t * QB:(t + 1) * QB], in_=ps,
                                         func=_EXP, scale=SCALE)
                # denominators: den[q, s] = sum_k expS[k, q]
                den = pden.tile([P, 2], F32, tag="den", name="den")
                for s in range(2):
                    for t in range(kt):
                        # N=1 violates fp32r ISA restrictions -> plain fp32
                        nc.tensor.matmul(
                            den[:, s:s + 1],
                            expS[:, t * QB + P * s: t * QB + P * (s + 1)]
                            .bitcast(F32),
                            ones_sb.bitcast(F32), start=t == 0,
                            stop=t == kt - 1, skip_group_check=True)
                recip = recip_pool.tile([P, 2], F32, tag="recip", name="recip")
                nc.vector.reciprocal(recip, den)
                # pass 2: ctx[q, d] = sum_k expS[k, q] * V[k, d], then normalize
                for s in range(2):
                    for n in range(2):
                        ps = p512.tile([P, 512], F32, tag="mm512", name="ps_c")
                        for t in range(kt):
                            mm(ps, expS[:, t * QB + P * s: t * QB + P * (s + 1)],
                               V_sb[:, t * D + 512 * n: t * D + 512 * (n + 1)],
                               t == 0, t == kt - 1)
                        ot = out_pool.tile([P, 512], F32, tag="out", name="ot")
                        nc.vector.tensor_scalar_mul(ot, ps, recip[:, s:s + 1])
                        nc.sync.dma_start(
                            out=out_d[QB * jb + P * s: QB * jb + P * (s + 1),
                                      512 * n: 512 * (n + 1)],
                            in_=ot)


def build_nc():
    nc = bacc.Bacc("TRN2", target_bir_lowering=False, debug=False, num_devices=8)
    xT_d = nc.dram_tensor("xT", [D, T], BF16, kind="ExternalInput")
    xTk_d = nc.dram_tensor("xTk", [D, T // 2], BF16, kind="ExternalInput")
    xTq_d = nc.dram_tensor("xTq", [D, T // 2], BF16, kind="ExternalInput")
    wq_d = nc.dram_tensor("wq", [D, D], BF16, kind="ExternalInput")
    wk_d = nc.dram_tensor("wk", [D, D], BF16, kind="ExternalInput")
    wv_d = nc.dram_tensor("wv", [D, D], BF16, kind="ExternalInput")
    masks_d = nc.dram_tensor("masks", [4, P, QB], F32, kind="ExternalInput")
    out_d = nc.dram_tensor("out", [T // 2, D], F32, kind="ExternalOutput")
    with tile.TileContext(nc) as tc:
        _emit(nc, tc, xT_d[:], xTk_d[:], xTq_d[:], wq_d[:], wk_d[:], wv_d[:],
              masks_d[:],
              out_d[:])
    nc.compile()
    return nc


def make_masks(h):
    """Additive causal mask: 0 where key (128u + p) <= query (2j + h), else
    -1e9, within a 512-position diagonal window (positions relative to the
    q-block base).  Applied to raw scores before exp."""
    u = np.arange(4)[:, None, None]
    p = np.arange(P)[None, :, None]
    j = np.arange(QB)[None, None, :]
    vis = (128 * u + p <= 2 * j + h)
    return np.where(vis, 0.0, MASK_NEG).astype(np.float32)


def make_in_maps(x, W_query, W_key, W_value):
    wq = np.ascontiguousarray(W_query).astype(BF16_NP)
    wk = np.ascontiguousarray(W_key).astype(BF16_NP)
    wv = np.ascontiguousarray(W_value).astype(BF16_NP)
    masks = [make_masks(h) for h in range(2)]
    in_maps = []
    for core in range(8):
        b, h = divmod(core, 2)
        xb = np.asarray(x[b], dtype=np.float32)
        in_maps.append({
            "xT": np.ascontiguousarray(xb.T).astype(BF16_NP),
            "xTk": np.ascontiguousarray(xb[1024 * h:1024 * (h + 1)].T)
                   .astype(BF16_NP),
            "xTq": np.ascontiguousarray(xb[h::2].T).astype(BF16_NP),
            "wq": wq, "wk": wk, "wv": wv,
            "masks": masks[h],
        })
    return in_maps


_NC_CACHE = {}
LAST_EXEC_NS = None


def kernel(x, W_query, W_key, W_value):
    global LAST_EXEC_NS
    from concourse.bass_utils import run_bass_kernel_spmd

    if "nc" not in _NC_CACHE:
        _NC_CACHE["nc"] = build_nc()
    nc = _NC_CACHE["nc"]

    in_maps = make_in_maps(x, W_query, W_key, W_value)
    trace = bool(os.environ.get("BASS_TRACE"))
    res = run_bass_kernel_spmd(nc, in_maps, core_ids=list(range(8)), trace=trace)
    LAST_EXEC_NS = res.exec_time_ns

    out = np.empty((B, T, D), dtype=np.float32)
    for core in range(8):
        b, h = divmod(core, 2)
        out[b, h::2, :] = res.results[core]["out"]
    return out


if __name__ == "__main__":
    import time
    t0 = time.time()
    nc = build_nc()
    print(f"build+compile took {time.time() - t0:.1f}s")
    n_inst = sum(len(getattr(e, 'instructions', [])) for e in nc.engines) \
        if hasattr(nc, 'engines') else -1
    print("built ok")


# revision 13
# speedup vs baseline: 1.3753x; 1.0853x over previous
"""Causal single-head attention on 8 Trainium2 NeuronCores.

Problem: x[4, 2048, 1024] fp32, Wq/Wk/Wv[1024, 1024] fp32.
  q,k,v = x@Wq, x@Wk, x@Wv ; out = softmax(mask(q k^T)/32) @ v

Sharding (SPMD — one program, 8 cores, per-core data):
  core = 2*b + h  handles batch b, queries {t : t % 2 == h} (1024 queries).
  The interleaved (mod-2) query split makes the causal block structure
  shape-identical across cores: per-core q-block jb (256 queries, spanning
  global positions [512*jb, 512*jb+512)) needs k-tiles 0..4*(jb+1)-1 on
  every core.  Causal masking inside the 4 diagonal k-tiles depends only on
  (u = t-4*jb, parity h) => 4 mask tiles passed as per-core data.

Layouts:
  - host passes x[b]^T ([D, T]) so Q^T/K^T/V all come out of matmuls with
    no on-chip transposes (contraction over d_in needs d_in on partitions).
  - scores are computed transposed ([k, q]) so that expS can feed the
    attn @ V matmul directly as the stationary operand (lhsT = expS[k, q]).
  - softmax denominator = ones-matmul over the key partition dim, and no
    max-subtraction (logits/32 are ~N(0, 0.41^2); exp never overflows).

Dtypes: bf16 matmul inputs for projections and scores (fp32 PSUM accum),
fp32 V / expS with float32r (full-rate fp32) context matmuls, fp32 softmax.
"""

import os
import numpy as np
import ml_dtypes

import concourse.bass as bass
import concourse.mybir as mybir
import concourse.tile as tile
from concourse import bacc

F32 = mybir.dt.float32
F32R = mybir.dt.float32r
BF16 = mybir.dt.bfloat16
BF16_NP = ml_dtypes.bfloat16

B, T, D = 4, 2048, 1024
P = 128
DC = D // P          # 8 contraction chunks
NW = T // 512        # 4 token windows (K/V projection granularity)
NT = T // P          # 16 key tiles
QB = 256             # queries per q-block (per core)
NJB = (T // 2) // QB # 4 q-blocks per core
SCALE = 1.0 / 32.0   # 1/sqrt(D)

# V / expS storage dtype: fp32 + float32r matmuls (accurate, full rate at
# N>=256).  Flip to BF16 if hardware shows fp32r matmuls are slow.
V_F32 = True
V_DT = F32R if V_F32 else BF16   # fp32r: walrus requires producers to round
MASK_NEG = -1.0e9
# Split the K^T projection across the core pair (each core projects its own
# 1024 tokens) and AllGather the halves; the 2 MB bf16 gather (~39 us) hides
# completely behind the full V + Q projections.  V stays locally projected —
# its 8 MB gather measured ~109 us and stalls the PE (tried, reverted).
K_SPLIT = True
PAIRS = [[0, 1], [2, 3], [4, 5], [6, 7]]
_EXP = mybir.ActivationFunctionType.Exp


def _emit(nc, tc, xT_d, xTk_d, xTq_d, wq_d, wk_d, wv_d, masks_d, out_d):
    HT = T // 2  # queries per core

    def mm(out, lhsT, rhs, start, stop, **kw):
        if out.dtype == F32 and lhsT.dtype == F32:
            lhsT = lhsT.bitcast(F32R)
            rhs = rhs.bitcast(F32R)
        nc.tensor.matmul(out, lhsT, rhs, start=start, stop=stop, **kw)

    with (
        tc.sbuf_pool(name="persist", bufs=1) as persist,
        tc.sbuf_pool(name="recipp", bufs=2) as recip_pool,
        tc.sbuf_pool(name="outp", bufs=4) as out_pool,
        tc.psum_pool(name="p512", bufs=4) as p512,
        tc.psum_pool(name="p256", bufs=3) as p256,
        tc.psum_pool(name="pden", bufs=1) as pden,
    ):
        # ---- persistent SBUF tensors ----
        K_sb = persist.tile([P, DC * T], BF16, tag="K", name="K_sb")
        V_sb = persist.tile([P, NT * D], V_DT, tag="V", name="V_sb")
        Q_sb = persist.tile([P, DC * HT], BF16, tag="Q", name="Q_sb")
        expS = persist.tile([P, NT * QB], V_DT, tag="E", name="expS")
        mask_sb = persist.tile([P, 4 * QB], F32, tag="M", name="mask_sb")
        ones_sb = persist.tile([P, 1], V_DT, tag="O", name="ones_sb")
        ones_f32 = persist.tile([P, 1], F32, tag="O32", name="ones_f32")
        nc.vector.memset(ones_f32, 1.0)
        nc.scalar.copy(out=ones_sb, in_=ones_f32)
        for u in range(4):
            nc.sync.dma_start(out=mask_sb[:, u * QB:(u + 1) * QB], in_=masks_d[u])

        # ---- projections: K^T (pair-split + AllGather) and V (local) ----
        with (
            tc.sbuf_pool(name="wkvp", bufs=1) as wkv_pool,
            tc.sbuf_pool(name="xtwp", bufs=2) as xtw_pool,
            tc.sbuf_pool(name="stgp", bufs=6) as stg_pool,
            tc.tile_pool(name="drp", bufs=1, space="DRAM") as dr_pool,
            nc.named_scope("kv_proj"),
        ):
            wk_sb = wkv_pool.tile([P, DC * D], BF16, tag="wk", name="wk_sb")
            wv_sb = wkv_pool.tile([P, DC * D], BF16, tag="wv", name="wv_sb")
            for c in range(DC):
                nc.sync.dma_start(out=wk_sb[:, c * D:(c + 1) * D],
                                  in_=wk_d[c * P:(c + 1) * P, :])
            if K_SPLIT:
                kloc = dr_pool.tile([D, T // 2], BF16, tag="kloc", name="kloc")
                kg = dr_pool.tile([2, D, T // 2], BF16, tag="kg", name="kg")
                # K^T of own token half first, so the gather launches early
                for w in range(NW // 2):
                    xtk = xtw_pool.tile([P, DC * 512], BF16, tag="xtw",
                                        name="xtk")
                    for c in range(DC):
                        nc.sync.dma_start(
                            out=xtk[:, c * 512:(c + 1) * 512],
                            in_=xTk_d[c * P:(c + 1) * P, 512 * w:512 * (w + 1)])
                    for c2 in range(DC):
                        ps = p512.tile([P, 512], F32, tag="mm512", name="ps_k")
                        for c in range(DC):
                            mm(ps, wk_sb[:, c * D + P * c2: c * D + P * (c2 + 1)],
                               xtk[:, c * 512:(c + 1) * 512], c == 0, c == DC - 1)
                        st = stg_pool.tile([P, 512], BF16, tag="stk", name="stk")
                        nc.scalar.copy(out=st, in_=ps)
                        nc.gpsimd.dma_start(
                            out=kloc[c2 * P:(c2 + 1) * P, 512 * w:512 * (w + 1)],
                            in_=st)
                nc.gpsimd.collective_compute(
                    "AllGather", mybir.AluOpType.bypass, replica_groups=PAIRS,
                    ins=[kloc[:]], outs=[kg[:]])
                for c in range(DC):
                    for r in range(2):
                        nc.sync.dma_start(
                            out=K_sb[:, c * T + 1024 * r: c * T + 1024 * (r + 1)],
                            in_=kg[r, c * P:(c + 1) * P, :])
            # V (full, local) — PE work here hides the K gather
            for c in range(DC):
                nc.sync.dma_start(out=wv_sb[:, c * D:(c + 1) * D],
                                  in_=wv_d[c * P:(c + 1) * P, :])
            for w in range(NW):
                xtw = xtw_pool.tile([P, DC * 512], BF16, tag="xtw", name="xtw")
                for c in range(DC):
                    nc.sync.dma_start(
                        out=xtw[:, c * 512:(c + 1) * 512],
                        in_=xT_d[c * P:(c + 1) * P, 512 * w:512 * (w + 1)])
                if not K_SPLIT:
                    for c2 in range(DC):
                        ps = p512.tile([P, 512], F32, tag="mm512", name="ps_k")
                        for c in range(DC):
                            mm(ps, wk_sb[:, c * D + P * c2: c * D + P * (c2 + 1)],
                               xtw[:, c * 512:(c + 1) * 512], c == 0, c == DC - 1)
                        nc.scalar.copy(
                            out=K_sb[:, c2 * T + 512 * w: c2 * T + 512 * (w + 1)],
                            in_=ps)
                for ts in range(4):
                    t = 4 * w + ts
                    for n in range(2):
                        ps = p512.tile([P, 512], F32, tag="mm512", name="ps_v")
                        for c in range(DC):
                            mm(ps, xtw[:, c * 512 + P * ts: c * 512 + P * (ts + 1)],
                               wv_sb[:, c * D + 512 * n: c * D + 512 * (n + 1)],
                               c == 0, c == DC - 1)
                        nc.scalar.copy(
                            out=V_sb[:, t * D + 512 * n: t * D + 512 * (n + 1)],
                            in_=ps)

        # ---- Q^T projection (own queries only), per q-block ----
        with (
            tc.sbuf_pool(name="wqp", bufs=1) as wq_pool,
            tc.sbuf_pool(name="xtqp", bufs=2) as xtq_pool,
            nc.named_scope("q_proj"),
        ):
            wq_sb = wq_pool.tile([P, DC * D], BF16, tag="wq", name="wq_sb")
            for c in range(DC):
                nc.sync.dma_start(out=wq_sb[:, c * D:(c + 1) * D],
                                  in_=wq_d[c * P:(c + 1) * P, :])
            for jp in range(NJB // 2):  # project two q-blocks at once (N=512)
                xtq = xtq_pool.tile([P, DC * 512], BF16, tag="xtq", name="xtq")
                for c in range(DC):
                    nc.sync.dma_start(
                        out=xtq[:, c * 512:(c + 1) * 512],
                        in_=xTq_d[c * P:(c + 1) * P, 512 * jp:512 * (jp + 1)])
                for c2 in range(DC):
                    ps = p512.tile([P, 512], F32, tag="mm512", name="ps_q")
                    for c in range(DC):
                        mm(ps, wq_sb[:, c * D + P * c2: c * D + P * (c2 + 1)],
                           xtq[:, c * 512:(c + 1) * 512], c == 0, c == DC - 1)
                    nc.scalar.copy(
                        out=Q_sb[:, c2 * HT + 512 * jp: c2 * HT + 512 * (jp + 1)],
                        in_=ps)

        # ---- attention, per q-block ----
        with nc.named_scope("attn"):
            for jb in range(NJB):
                kt = 4 * (jb + 1)  # k-tiles needed by this q-block
                # pass 1: scores^T -> exp (-> mask on the 4 diagonal tiles)
                for t in range(kt):
                    ps = p256.tile([P, QB], F32, tag="mm256", name="ps_s")
                    for c in range(DC):
                        mm(ps, K_sb[:, c * T + P * t: c * T + P * (t + 1)],
                           Q_sb[:, c * HT + QB * jb: c * HT + QB * (jb + 1)],
                           c == 0, c == DC - 1)
                    if t >= kt - 4:
                        u = t - (kt - 4)
                        nc.vector.tensor_add(ps, ps,
                                             mask_sb[:, u * QB:(u + 1) * QB])
                    nc.scalar.activation(out=expS[:, t * QB:(t + 1) * QB], in_=ps,
                                         func=_EXP, scale=SCALE)
                # denominators: den[q, s] = sum_k expS[k, q]
                den = pden.tile([P, 2], F32, tag="den", name="den")
                for s in range(2):
                    for t in range(kt):
                        # N=1 violates fp32r ISA restrictions -> plain fp32
                        nc.tensor.matmul(
                            den[:, s:s + 1],
                            expS[:, t * QB + P * s: t * QB + P * (s + 1)]
                            .bitcast(F32),
                            ones_sb.bitcast(F32), start=t == 0,
                            stop=t == kt - 1, skip_group_check=True)
                recip = recip_pool.tile([P, 2], F32, tag="recip", name="recip")
                nc.vector.reciprocal(recip, den)
                # pass 2: ctx[q, d] = sum_k expS[k, q] * V[k, d], then normalize
                for s in range(2):
                    for n in range(2):
                        ps = p512.tile([P, 512], F32, tag="mm512", name="ps_c")
                        for t in range(kt):
                            mm(ps, expS[:, t * QB + P * s: t * QB + P * (s + 1)],
                               V_sb[:, t * D + 512 * n: t * D + 512 * (n + 1)],
                               t == 0, t == kt - 1)
                        ot = out_pool.tile([P, 512], F32, tag="out", name="ot")
                        nc.vector.tensor_scalar_mul(ot, ps, recip[:, s:s + 1])
                        nc.sync.dma_start(
                            out=out_d[QB * jb + P * s: QB * jb + P * (s + 1),
                                      512 * n: 512 * (n + 1)],
                            in_=ot)


def build_nc():
    nc = bacc.Bacc("TRN2", target_bir_lowering=False, debug=False, num_devices=8)
    xT_d = nc.dram_tensor("xT", [D, T], BF16, kind="ExternalInput")
    xTk_d = nc.dram_tensor("xTk", [D, T // 2], BF16, kind="ExternalInput")
    xTq_d = nc.dram_tensor("xTq", [D, T // 2], BF16, kind="ExternalInput")
    wq_d = nc.dram_tensor("wq", [D, D], BF16, kind="ExternalInput")
    wk_d = nc.dram_tensor("wk", [D, D], BF16, kind="ExternalInput")
    wv_d = nc.dram_tensor("wv", [D, D], BF16, kind="ExternalInput")
    masks_d = nc.dram_tensor("masks", [4, P, QB], F32, kind="ExternalInput")
    out_d = nc.dram_tensor("out", [T // 2, D], F32, kind="ExternalOutput")
    with tile.TileContext(nc) as tc:
        _emit(nc, tc, xT_d[:], xTk_d[:], xTq_d[:], wq_d[:], wk_d[:], wv_d[:],
              masks_d[:],
              out_d[:])
    nc.compile()
    return nc


def make_masks(h):
    """Additive causal mask: 0 where key (128u + p) <= query (2j + h), else
    -1e9, within a 512-position diagonal window (positions relative to the
    q-block base).  Applied to raw scores before exp."""
    u = np.arange(4)[:, None, None]
    p = np.arange(P)[None, :, None]
    j = np.arange(QB)[None, None, :]
    vis = (128 * u + p <= 2 * j + h)
    return np.where(vis, 0.0, MASK_NEG).astype(np.float32)


def make_in_maps(x, W_query, W_key, W_value):
    wq = np.ascontiguousarray(W_query).astype(BF16_NP)
    wk = np.ascontiguousarray(W_key).astype(BF16_NP)
    wv = np.ascontiguousarray(W_value).astype(BF16_NP)
    masks = [make_masks(h) for h in range(2)]
    in_maps = []
    for core in range(8):
        b, h = divmod(core, 2)
        xb = np.asarray(x[b], dtype=np.float32)
        in_maps.append({
            "xT": np.ascontiguousarray(xb.T).astype(BF16_NP),
            "xTk": np.ascontiguousarray(xb[1024 * h:1024 * (h + 1)].T)
                   .astype(BF16_NP),
            "xTq": np.ascontiguousarray(xb[h::2].T).astype(BF16_NP),
            "wq": wq, "wk": wk, "wv": wv,
            "masks": masks[h],
        })
    return in_maps


_NC_CACHE = {}
LAST_EXEC_NS = None


def kernel(x, W_query, W_key, W_value):
    global LAST_EXEC_NS
    from concourse.bass_utils import run_bass_kernel_spmd

    if "nc" not in _NC_CACHE:
        _NC_CACHE["nc"] = build_nc()
    nc = _NC_CACHE["nc"]

    in_maps = make_in_maps(x, W_query, W_key, W_value)
    trace = bool(os.environ.get("BASS_TRACE"))
    res = run_bass_kernel_spmd(nc, in_maps, core_ids=list(range(8)), trace=trace)
    LAST_EXEC_NS = res.exec_time_ns

    out = np.empty((B, T, D), dtype=np.float32)
    for core in range(8):
        b, h = divmod(core, 2)
        out[b, h::2, :] = res.results[core]["out"]
    return out


if __name__ == "__main__":
    import time
    t0 = time.time()
    nc = build_nc()
    print(f"build+compile took {time.time() - t0:.1f}s")
    n_inst = sum(len(getattr(e, 'instructions', [])) for e in nc.engines) \
        if hasattr(nc, 'engines') else -1
    print("built ok")


# revision 14
# speedup vs baseline: 1.4090x; 1.0245x over previous
"""Causal single-head attention on 8 Trainium2 NeuronCores.

Problem: x[4, 2048, 1024] fp32, Wq/Wk/Wv[1024, 1024] fp32.
  q,k,v = x@Wq, x@Wk, x@Wv ; out = softmax(mask(q k^T)/32) @ v

Sharding (SPMD — one program, 8 cores, per-core data):
  core = 2*b + h  handles batch b, queries {t : t % 2 == h} (1024 queries).
  The interleaved (mod-2) query split makes the causal block structure
  shape-identical across cores: per-core q-block jb (256 queries, spanning
  global positions [512*jb, 512*jb+512)) needs k-tiles 0..4*(jb+1)-1 on
  every core.  Causal masking inside the 4 diagonal k-tiles depends only on
  (u = t-4*jb, parity h) => 4 mask tiles passed as per-core data.

Layouts:
  - host passes x[b]^T ([D, T]) so Q^T/K^T/V all come out of matmuls with
    no on-chip transposes (contraction over d_in needs d_in on partitions).
  - scores are computed transposed ([k, q]) so that expS can feed the
    attn @ V matmul directly as the stationary operand (lhsT = expS[k, q]).
  - softmax denominator = ones-matmul over the key partition dim, and no
    max-subtraction (logits/32 are ~N(0, 0.41^2); exp never overflows).

Dtypes: bf16 matmul inputs for projections and scores (fp32 PSUM accum),
fp32 V / expS with float32r (full-rate fp32) context matmuls, fp32 softmax.
"""

import os
import numpy as np
import ml_dtypes

import concourse.bass as bass
import concourse.mybir as mybir
import concourse.tile as tile
from concourse import bacc

F32 = mybir.dt.float32
F32R = mybir.dt.float32r
BF16 = mybir.dt.bfloat16
BF16_NP = ml_dtypes.bfloat16

B, T, D = 4, 2048, 1024
P = 128
DC = D // P          # 8 contraction chunks
NW = T // 512        # 4 token windows (K/V projection granularity)
NT = T // P          # 16 key tiles
QB = 256             # queries per q-block (per core)
NJB = (T // 2) // QB # 4 q-blocks per core
SCALE = 1.0 / 32.0   # 1/sqrt(D)

# V / expS storage dtype: fp32 + float32r matmuls (accurate, full rate at
# N>=256).  Flip to BF16 if hardware shows fp32r matmuls are slow.
V_F32 = True
V_DT = F32R if V_F32 else BF16   # fp32r: walrus requires producers to round
MASK_NEG = -1.0e9
# Split the K^T projection across the core pair (each core projects its own
# 1024 tokens) and AllGather the halves; the 2 MB bf16 gather (~39 us) hides
# completely behind the full V + Q projections.  V stays locally projected —
# its 8 MB gather measured ~109 us and stalls the PE (tried, reverted).
K_SPLIT = True
PAIRS = [[0, 1], [2, 3], [4, 5], [6, 7]]
_EXP = mybir.ActivationFunctionType.Exp


def _emit(nc, tc, xT_d, xTk_d, xTq_d, wq_d, wk_d, wv_d, masks_d, out_d):
    HT = T // 2  # queries per core

    def mm(out, lhsT, rhs, start, stop, **kw):
        if out.dtype == F32 and lhsT.dtype == F32:
            lhsT = lhsT.bitcast(F32R)
            rhs = rhs.bitcast(F32R)
        nc.tensor.matmul(out, lhsT, rhs, start=start, stop=stop, **kw)

    with (
        tc.sbuf_pool(name="persist", bufs=1) as persist,
        tc.sbuf_pool(name="recipp", bufs=2) as recip_pool,
        tc.sbuf_pool(name="accp", bufs=2) as acc_pool,
        tc.sbuf_pool(name="outp", bufs=4) as out_pool,
        tc.psum_pool(name="p512", bufs=4) as p512,
        tc.psum_pool(name="p256", bufs=3) as p256,
        tc.psum_pool(name="pden", bufs=1) as pden,
    ):
        # ---- persistent SBUF tensors ----
        K_sb = persist.tile([P, DC * T], BF16, tag="K", name="K_sb")
        V_sb = persist.tile([P, NT * D], V_DT, tag="V", name="V_sb")
        Q_sb = persist.tile([P, DC * HT], BF16, tag="Q", name="Q_sb")
        expS = persist.tile([P, NT * QB], V_DT, tag="E", name="expS")
        mask_sb = persist.tile([P, 4 * QB], F32, tag="M", name="mask_sb")
        ones_f32 = persist.tile([P, 1], F32, tag="O32", name="ones_f32")
        nc.vector.memset(ones_f32, 1.0)
        for u in range(4):
            nc.sync.dma_start(out=mask_sb[:, u * QB:(u + 1) * QB], in_=masks_d[u])

        # ---- projections: K^T (pair-split + AllGather) and V (local) ----
        with (
            tc.sbuf_pool(name="wkvp", bufs=1) as wkv_pool,
            tc.sbuf_pool(name="xtwp", bufs=2) as xtw_pool,
            tc.sbuf_pool(name="stgp", bufs=6) as stg_pool,
            tc.tile_pool(name="drp", bufs=1, space="DRAM") as dr_pool,
            nc.named_scope("kv_proj"),
        ):
            wk_sb = wkv_pool.tile([P, DC * D], BF16, tag="wk", name="wk_sb")
            wv_sb = wkv_pool.tile([P, DC * D], BF16, tag="wv", name="wv_sb")
            for c in range(DC):
                nc.sync.dma_start(out=wk_sb[:, c * D:(c + 1) * D],
                                  in_=wk_d[c * P:(c + 1) * P, :])
            if K_SPLIT:
                kloc = dr_pool.tile([D, T // 2], BF16, tag="kloc", name="kloc")
                kg = dr_pool.tile([2, D, T // 2], BF16, tag="kg", name="kg")
                # K^T of own token half first, so the gather launches early
                for w in range(NW // 2):
                    xtk = xtw_pool.tile([P, DC * 512], BF16, tag="xtw",
                                        name="xtk")
                    for c in range(DC):
                        nc.sync.dma_start(
                            out=xtk[:, c * 512:(c + 1) * 512],
                            in_=xTk_d[c * P:(c + 1) * P, 512 * w:512 * (w + 1)])
                    for c2 in range(DC):
                        ps = p512.tile([P, 512], F32, tag="mm512", name="ps_k")
                        for c in range(DC):
                            mm(ps, wk_sb[:, c * D + P * c2: c * D + P * (c2 + 1)],
                               xtk[:, c * 512:(c + 1) * 512], c == 0, c == DC - 1)
                        st = stg_pool.tile([P, 512], BF16, tag="stk", name="stk")
                        nc.scalar.copy(out=st, in_=ps)
                        nc.sync.dma_start(
                            out=kloc[c2 * P:(c2 + 1) * P, 512 * w:512 * (w + 1)],
                            in_=st)
                nc.gpsimd.collective_compute(
                    "AllGather", mybir.AluOpType.bypass, replica_groups=PAIRS,
                    ins=[kloc[:]], outs=[kg[:]])
                for c in range(DC):
                    for r in range(2):
                        nc.sync.dma_start(
                            out=K_sb[:, c * T + 1024 * r: c * T + 1024 * (r + 1)],
                            in_=kg[r, c * P:(c + 1) * P, :])
            # V (full, local) — PE work here hides the K gather
            for c in range(DC):
                nc.sync.dma_start(out=wv_sb[:, c * D:(c + 1) * D],
                                  in_=wv_d[c * P:(c + 1) * P, :])
            for w in range(NW):
                xtw = xtw_pool.tile([P, DC * 512], BF16, tag="xtw", name="xtw")
                for c in range(DC):
                    nc.sync.dma_start(
                        out=xtw[:, c * 512:(c + 1) * 512],
                        in_=xT_d[c * P:(c + 1) * P, 512 * w:512 * (w + 1)])
                if not K_SPLIT:
                    for c2 in range(DC):
                        ps = p512.tile([P, 512], F32, tag="mm512", name="ps_k")
                        for c in range(DC):
                            mm(ps, wk_sb[:, c * D + P * c2: c * D + P * (c2 + 1)],
                               xtw[:, c * 512:(c + 1) * 512], c == 0, c == DC - 1)
                        nc.scalar.copy(
                            out=K_sb[:, c2 * T + 512 * w: c2 * T + 512 * (w + 1)],
                            in_=ps)
                for ts in range(4):
                    t = 4 * w + ts
                    for n in range(2):
                        ps = p512.tile([P, 512], F32, tag="mm512", name="ps_v")
                        for c in range(DC):
                            mm(ps, xtw[:, c * 512 + P * ts: c * 512 + P * (ts + 1)],
                               wv_sb[:, c * D + 512 * n: c * D + 512 * (n + 1)],
                               c == 0, c == DC - 1)
                        nc.scalar.copy(
                            out=V_sb[:, t * D + 512 * n: t * D + 512 * (n + 1)],
                            in_=ps)

        # ---- Q^T projection (own queries only), per q-block ----
        with (
            tc.sbuf_pool(name="wqp", bufs=1) as wq_pool,
            tc.sbuf_pool(name="xtqp", bufs=2) as xtq_pool,
            nc.named_scope("q_proj"),
        ):
            wq_sb = wq_pool.tile([P, DC * D], BF16, tag="wq", name="wq_sb")
            for c in range(DC):
                nc.sync.dma_start(out=wq_sb[:, c * D:(c + 1) * D],
                                  in_=wq_d[c * P:(c + 1) * P, :])
            for jp in range(NJB // 2):  # project two q-blocks at once (N=512)
                xtq = xtq_pool.tile([P, DC * 512], BF16, tag="xtq", name="xtq")
                for c in range(DC):
                    nc.sync.dma_start(
                        out=xtq[:, c * 512:(c + 1) * 512],
                        in_=xTq_d[c * P:(c + 1) * P, 512 * jp:512 * (jp + 1)])
                for c2 in range(DC):
                    ps = p512.tile([P, 512], F32, tag="mm512", name="ps_q")
                    for c in range(DC):
                        mm(ps, wq_sb[:, c * D + P * c2: c * D + P * (c2 + 1)],
                           xtq[:, c * 512:(c + 1) * 512], c == 0, c == DC - 1)
                    nc.scalar.copy(
                        out=Q_sb[:, c2 * HT + 512 * jp: c2 * HT + 512 * (jp + 1)],
                        in_=ps)

        # ---- attention, per q-block ----
        with nc.named_scope("attn"):
            for jb in range(NJB):
                kt = 4 * (jb + 1)  # k-tiles needed by this q-block
                # pass 1: scores^T -> exp (-> mask on the 4 diagonal tiles)
                for t in range(kt):
                    ps = p256.tile([P, QB], F32, tag="mm256", name="ps_s")
                    for c in range(DC):
                        mm(ps, K_sb[:, c * T + P * t: c * T + P * (t + 1)],
                           Q_sb[:, c * HT + QB * jb: c * HT + QB * (jb + 1)],
                           c == 0, c == DC - 1)
                    if t >= kt - 4:
                        u = t - (kt - 4)
                        nc.vector.tensor_add(ps, ps,
                                             mask_sb[:, u * QB:(u + 1) * QB])
                    nc.scalar.activation(out=expS[:, t * QB:(t + 1) * QB], in_=ps,
                                         func=_EXP, scale=SCALE)
                # denominators: den[q, s] = sum_k expS[k, q].  Partition-
                # partial sums accumulate on the (idle) DVE; one tiny fp32
                # matmul per q-sub does the final cross-partition reduction
                # (N=1 fp32 matmuls are slow on the PE, ~220ns each).
                acc = acc_pool.tile([P, QB], F32, tag="acc", name="acc")
                nc.vector.tensor_copy(acc, expS[:, 0:QB].bitcast(F32))
                for t in range(1, kt):
                    nc.vector.tensor_add(
                        acc, acc, expS[:, t * QB:(t + 1) * QB].bitcast(F32))
                den = pden.tile([P, 2], F32, tag="den", name="den")
                for s in range(2):
                    nc.tensor.matmul(den[:, s:s + 1],
                                     acc[:, P * s:P * (s + 1)], ones_f32,
                                     start=True, stop=True,
                                     skip_group_check=True)
                recip = recip_pool.tile([P, 2], F32, tag="recip", name="recip")
                nc.vector.reciprocal(recip, den)
                # pass 2: ctx[q, d] = sum_k expS[k, q] * V[k, d], then normalize
                for s in range(2):
                    for n in range(2):
                        ps = p512.tile([P, 512], F32, tag="mm512", name="ps_c")
                        for t in range(kt):
                            mm(ps, expS[:, t * QB + P * s: t * QB + P * (s + 1)],
                               V_sb[:, t * D + 512 * n: t * D + 512 * (n + 1)],
                               t == 0, t == kt - 1)
                        ot = out_pool.tile([P, 512], F32, tag="out", name="ot")
                        nc.vector.tensor_scalar_mul(ot, ps, recip[:, s:s + 1])
                        nc.sync.dma_start(
                            out=out_d[QB * jb + P * s: QB * jb + P * (s + 1),
                                      512 * n: 512 * (n + 1)],
                            in_=ot)


def build_nc():
    nc = bacc.Bacc("TRN2", target_bir_lowering=False, debug=False, num_devices=8)
    xT_d = nc.dram_tensor("xT", [D, T], BF16, kind="ExternalInput")
    xTk_d = nc.dram_tensor("xTk", [D, T // 2], BF16, kind="ExternalInput")
    xTq_d = nc.dram_tensor("xTq", [D, T // 2], BF16, kind="ExternalInput")
    wq_d = nc.dram_tensor("wq", [D, D], BF16, kind="ExternalInput")
    wk_d = nc.dram_tensor("wk", [D, D], BF16, kind="ExternalInput")
    wv_d = nc.dram_tensor("wv", [D, D], BF16, kind="ExternalInput")
    masks_d = nc.dram_tensor("masks", [4, P, QB], F32, kind="ExternalInput")
    out_d = nc.dram_tensor("out", [T // 2, D], F32, kind="ExternalOutput")
    with tile.TileContext(nc) as tc:
        _emit(nc, tc, xT_d[:], xTk_d[:], xTq_d[:], wq_d[:], wk_d[:], wv_d[:],
              masks_d[:],
              out_d[:])
    nc.compile()
    return nc


def make_masks(h):
    """Additive causal mask: 0 where key (128u + p) <= query (2j + h), else
    -1e9, within a 512-position diagonal window (positions relative to the
    q-block base).  Applied to raw scores before exp."""
    u = np.arange(4)[:, None, None]
    p = np.arange(P)[None, :, None]
    j = np.arange(QB)[None, None, :]
    vis = (128 * u + p <= 2 * j + h)
    return np.where(vis, 0.0, MASK_NEG).astype(np.float32)


def make_in_maps(x, W_query, W_key, W_value):
    wq = np.ascontiguousarray(W_query).astype(BF16_NP)
    wk = np.ascontiguousarray(W_key).astype(BF16_NP)
    wv = np.ascontiguousarray(W_value).astype(BF16_NP)
    masks = [make_masks(h) for h in range(2)]
    in_maps = []
    for core in range(8):
        b, h = divmod(core, 2)
        xb = np.asarray(x[b], dtype=np.float32)
        in_maps.append({
            "xT": np.ascontiguousarray(xb.T).astype(BF16_NP),
            "xTk": np.ascontiguousarray(xb[1024 * h:1024 * (h + 1)].T)
                   .astype(BF16_NP),
            "xTq": np.ascontiguousarray(xb[h::2].T).astype(BF16_NP),
            "wq": wq, "wk": wk, "wv": wv,
            "masks": masks[h],
        })
    return in_maps


_NC_CACHE = {}
LAST_EXEC_NS = None


def kernel(x, W_query, W_key, W_value):
    global LAST_EXEC_NS
    from concourse.bass_utils import run_bass_kernel_spmd

    if "nc" not in _NC_CACHE:
        _NC_CACHE["nc"] = build_nc()
    nc = _NC_CACHE["nc"]

    in_maps = make_in_maps(x, W_query, W_key, W_value)
    trace = bool(os.environ.get("BASS_TRACE"))
    res = run_bass_kernel_spmd(nc, in_maps, core_ids=list(range(8)), trace=trace)
    LAST_EXEC_NS = res.exec_time_ns

    out = np.empty((B, T, D), dtype=np.float32)
    for core in range(8):
        b, h = divmod(core, 2)
        out[b, h::2, :] = res.results[core]["out"]
    return out


if __name__ == "__main__":
    import time
    t0 = time.time()
    nc = build_nc()
    print(f"build+compile took {time.time() - t0:.1f}s")
    n_inst = sum(len(getattr(e, 'instructions', [])) for e in nc.engines) \
        if hasattr(nc, 'engines') else -1
    print("built ok")


# revision 15
# speedup vs baseline: 1.4992x; 1.0640x over previous
"""Causal single-head attention on 8 Trainium2 NeuronCores.

Problem: x[4, 2048, 1024] fp32, Wq/Wk/Wv[1024, 1024] fp32.
  q,k,v = x@Wq, x@Wk, x@Wv ; out = softmax(mask(q k^T)/32) @ v

Sharding (SPMD — one program, 8 cores, per-core data):
  core = 2*b + h  handles batch b, queries {t : t % 2 == h} (1024 queries).
  The interleaved (mod-2) query split makes the causal block structure
  shape-identical across cores: per-core q-block jb (256 queries, spanning
  global positions [512*jb, 512*jb+512)) needs k-tiles 0..4*(jb+1)-1 on
  every core.  Causal masking inside the 4 diagonal k-tiles depends only on
  (u = t-4*jb, parity h) => 4 mask tiles passed as per-core data.

Layouts:
  - host passes x[b]^T ([D, T]) so Q^T/K^T/V all come out of matmuls with
    no on-chip transposes (contraction over d_in needs d_in on partitions).
  - scores are computed transposed ([k, q]) so that expS can feed the
    attn @ V matmul directly as the stationary operand (lhsT = expS[k, q]).
  - softmax denominator = ones-matmul over the key partition dim, and no
    max-subtraction (logits/32 are ~N(0, 0.41^2); exp never overflows).

Dtypes: bf16 matmul inputs for projections and scores (fp32 PSUM accum),
fp32 V / expS with float32r (full-rate fp32) context matmuls, fp32 softmax.
"""

import os
import numpy as np
import ml_dtypes

import concourse.bass as bass
import concourse.mybir as mybir
import concourse.tile as tile
from concourse import bacc

F32 = mybir.dt.float32
F32R = mybir.dt.float32r
BF16 = mybir.dt.bfloat16
BF16_NP = ml_dtypes.bfloat16

B, T, D = 4, 2048, 1024
P = 128
DC = D // P          # 8 contraction chunks
NW = T // 512        # 4 token windows (K/V projection granularity)
NT = T // P          # 16 key tiles
QB = 256             # queries per q-block (per core)
NJB = (T // 2) // QB # 4 q-blocks per core
SCALE = 1.0 / 32.0   # 1/sqrt(D)

# V / expS storage dtype: fp32 + float32r matmuls (accurate, full rate at
# N>=256).  Flip to BF16 if hardware shows fp32r matmuls are slow.
V_F32 = True
V_DT = F32R if V_F32 else BF16   # fp32r: walrus requires producers to round
MASK_NEG = -1.0e9
# Split the K^T projection across the core pair (each core projects its own
# 1024 tokens) and AllGather the halves; the 2 MB bf16 gather (~39 us) hides
# completely behind the full V + Q projections.  V stays locally projected —
# its 8 MB gather measured ~109 us and stalls the PE (tried, reverted).
K_SPLIT = True
PAIRS = [[0, 1], [2, 3], [4, 5], [6, 7]]
_EXP = mybir.ActivationFunctionType.Exp


def _emit(nc, tc, xT_d, xTk_d, xTq_d, wq_d, wk_d, wv_d, masks_d, out_d):
    HT = T // 2  # queries per core

    def mm(out, lhsT, rhs, start, stop, **kw):
        if out.dtype == F32 and lhsT.dtype == F32:
            lhsT = lhsT.bitcast(F32R)
            rhs = rhs.bitcast(F32R)
        nc.tensor.matmul(out, lhsT, rhs, start=start, stop=stop, **kw)

    with (
        tc.sbuf_pool(name="persist", bufs=1) as persist,
        tc.sbuf_pool(name="recipp", bufs=2) as recip_pool,
        tc.sbuf_pool(name="accp", bufs=2) as acc_pool,
        tc.sbuf_pool(name="outp", bufs=4) as out_pool,
        tc.psum_pool(name="p512", bufs=4) as p512,
        tc.psum_pool(name="p256", bufs=3) as p256,
        tc.psum_pool(name="pden", bufs=1) as pden,
    ):
        # ---- persistent SBUF tensors ----
        K_sb = persist.tile([P, DC * T], BF16, tag="K", name="K_sb")
        V_sb = persist.tile([P, NT * D], V_DT, tag="V", name="V_sb")
        Q_sb = persist.tile([P, DC * HT], BF16, tag="Q", name="Q_sb")
        expS = persist.tile([P, NT * QB], V_DT, tag="E", name="expS")
        mask_sb = persist.tile([P, 4 * QB], F32, tag="M", name="mask_sb")
        ones_f32 = persist.tile([P, 1], F32, tag="O32", name="ones_f32")
        nc.vector.memset(ones_f32, 1.0)
        for u in range(4):
            nc.sync.dma_start(out=mask_sb[:, u * QB:(u + 1) * QB], in_=masks_d[u])

        # ---- projections: K^T (pair-split + AllGather) and V (local) ----
        with (
            tc.sbuf_pool(name="wkvp", bufs=1) as wkv_pool,
            tc.sbuf_pool(name="xtwp", bufs=2) as xtw_pool,
            tc.sbuf_pool(name="stgp", bufs=6) as stg_pool,
            tc.tile_pool(name="drp", bufs=1, space="DRAM") as dr_pool,
            nc.named_scope("kv_proj"),
        ):
            wk_sb = wkv_pool.tile([P, DC * D], BF16, tag="wk", name="wk_sb")
            wv_sb = wkv_pool.tile([P, DC * D], BF16, tag="wv", name="wv_sb")
            for c in range(DC):
                nc.sync.dma_start(out=wk_sb[:, c * D:(c + 1) * D],
                                  in_=wk_d[c * P:(c + 1) * P, :])
            if K_SPLIT:
                # K^T of own token half first; one pipelined AllGather per
                # 512-token window so gather #0 launches while window 1 is
                # still projecting (pair gathers have ~20us launch latency).
                klocs, kgs = [], []
                for w in range(NW // 2):
                    klocs.append(dr_pool.tile([D, 512], BF16, tag=f"kloc{w}",
                                              name=f"kloc{w}"))
                    kgs.append(dr_pool.tile([2, D, 512], BF16, tag=f"kg{w}",
                                            name=f"kg{w}"))
                for w in range(NW // 2):
                    xtk = xtw_pool.tile([P, DC * 512], BF16, tag="xtw",
                                        name="xtk")
                    for c in range(DC):
                        nc.sync.dma_start(
                            out=xtk[:, c * 512:(c + 1) * 512],
                            in_=xTk_d[c * P:(c + 1) * P, 512 * w:512 * (w + 1)])
                    for c2 in range(DC):
                        ps = p512.tile([P, 512], F32, tag="mm512", name="ps_k")
                        for c in range(DC):
                            mm(ps, wk_sb[:, c * D + P * c2: c * D + P * (c2 + 1)],
                               xtk[:, c * 512:(c + 1) * 512], c == 0, c == DC - 1)
                        st = stg_pool.tile([P, 512], BF16, tag="stk", name="stk")
                        nc.scalar.copy(out=st, in_=ps)
                        nc.sync.dma_start(
                            out=klocs[w][c2 * P:(c2 + 1) * P, :], in_=st)
                    nc.gpsimd.collective_compute(
                        "AllGather", mybir.AluOpType.bypass,
                        replica_groups=PAIRS, ins=[klocs[w][:]],
                        outs=[kgs[w][:]])
                for lw in range(NW // 2):
                    for r in range(2):
                        gw = 2 * r + lw  # global token window
                        for c in range(DC):
                            nc.sync.dma_start(
                                out=K_sb[:, c * T + 512 * gw:
                                         c * T + 512 * (gw + 1)],
                                in_=kgs[lw][r, c * P:(c + 1) * P, :])
            # V (full, local) — PE work here hides the K gather
            for c in range(DC):
                nc.sync.dma_start(out=wv_sb[:, c * D:(c + 1) * D],
                                  in_=wv_d[c * P:(c + 1) * P, :])
            for w in range(NW):
                xtw = xtw_pool.tile([P, DC * 512], BF16, tag="xtw", name="xtw")
                for c in range(DC):
                    nc.sync.dma_start(
                        out=xtw[:, c * 512:(c + 1) * 512],
                        in_=xT_d[c * P:(c + 1) * P, 512 * w:512 * (w + 1)])
                if not K_SPLIT:
                    for c2 in range(DC):
                        ps = p512.tile([P, 512], F32, tag="mm512", name="ps_k")
                        for c in range(DC):
                            mm(ps, wk_sb[:, c * D + P * c2: c * D + P * (c2 + 1)],
                               xtw[:, c * 512:(c + 1) * 512], c == 0, c == DC - 1)
                        nc.scalar.copy(
                            out=K_sb[:, c2 * T + 512 * w: c2 * T + 512 * (w + 1)],
                            in_=ps)
                for ts in range(4):
                    t = 4 * w + ts
                    for n in range(2):
                        ps = p512.tile([P, 512], F32, tag="mm512", name="ps_v")
                        for c in range(DC):
                            mm(ps, xtw[:, c * 512 + P * ts: c * 512 + P * (ts + 1)],
                               wv_sb[:, c * D + 512 * n: c * D + 512 * (n + 1)],
                               c == 0, c == DC - 1)
                        nc.scalar.copy(
                            out=V_sb[:, t * D + 512 * n: t * D + 512 * (n + 1)],
                            in_=ps)

        # ---- Q^T projection (own queries only), per q-block ----
        with (
            tc.sbuf_pool(name="wqp", bufs=1) as wq_pool,
            tc.sbuf_pool(name="xtqp", bufs=2) as xtq_pool,
            nc.named_scope("q_proj"),
        ):
            wq_sb = wq_pool.tile([P, DC * D], BF16, tag="wq", name="wq_sb")
            for c in range(DC):
                nc.sync.dma_start(out=wq_sb[:, c * D:(c + 1) * D],
                                  in_=wq_d[c * P:(c + 1) * P, :])
            for jp in range(NJB // 2):  # project two q-blocks at once (N=512)
                xtq = xtq_pool.tile([P, DC * 512], BF16, tag="xtq", name="xtq")
                for c in range(DC):
                    nc.sync.dma_start(
                        out=xtq[:, c * 512:(c + 1) * 512],
                        in_=xTq_d[c * P:(c + 1) * P, 512 * jp:512 * (jp + 1)])
                for c2 in range(DC):
                    ps = p512.tile([P, 512], F32, tag="mm512", name="ps_q")
                    for c in range(DC):
                        mm(ps, wq_sb[:, c * D + P * c2: c * D + P * (c2 + 1)],
                           xtq[:, c * 512:(c + 1) * 512], c == 0, c == DC - 1)
                    nc.scalar.copy(
                        out=Q_sb[:, c2 * HT + 512 * jp: c2 * HT + 512 * (jp + 1)],
                        in_=ps)

        # ---- attention, per q-block ----
        with nc.named_scope("attn"):
            for jb in range(NJB):
                kt = 4 * (jb + 1)  # k-tiles needed by this q-block
                # pass 1: scores^T -> exp (-> mask on the 4 diagonal tiles)
                for t in range(kt):
                    ps = p256.tile([P, QB], F32, tag="mm256", name="ps_s")
                    for c in range(DC):
                        mm(ps, K_sb[:, c * T + P * t: c * T + P * (t + 1)],
                           Q_sb[:, c * HT + QB * jb: c * HT + QB * (jb + 1)],
                           c == 0, c == DC - 1)
                    if t >= kt - 4:
                        u = t - (kt - 4)
                        nc.vector.tensor_add(ps, ps,
                                             mask_sb[:, u * QB:(u + 1) * QB])
                    nc.scalar.activation(out=expS[:, t * QB:(t + 1) * QB], in_=ps,
                                         func=_EXP, scale=SCALE)
                # denominators: den[q, s] = sum_k expS[k, q].  Partition-
                # partial sums accumulate on the (idle) DVE; one tiny fp32
                # matmul per q-sub does the final cross-partition reduction
                # (N=1 fp32 matmuls are slow on the PE, ~220ns each).
                acc = acc_pool.tile([P, QB], F32, tag="acc", name="acc")
                nc.vector.tensor_copy(acc, expS[:, 0:QB].bitcast(F32))
                for t in range(1, kt):
                    nc.vector.tensor_add(
                        acc, acc, expS[:, t * QB:(t + 1) * QB].bitcast(F32))
                den = pden.tile([P, 2], F32, tag="den", name="den")
                for s in range(2):
                    nc.tensor.matmul(den[:, s:s + 1],
                                     acc[:, P * s:P * (s + 1)], ones_f32,
                                     start=True, stop=True,
                                     skip_group_check=True)
                recip = recip_pool.tile([P, 2], F32, tag="recip", name="recip")
                nc.vector.reciprocal(recip, den)
                # pass 2: ctx[q, d] = sum_k expS[k, q] * V[k, d], then normalize
                for s in range(2):
                    for n in range(2):
                        ps = p512.tile([P, 512], F32, tag="mm512", name="ps_c")
                        for t in range(kt):
                            mm(ps, expS[:, t * QB + P * s: t * QB + P * (s + 1)],
                               V_sb[:, t * D + 512 * n: t * D + 512 * (n + 1)],
                               t == 0, t == kt - 1)
                        ot = out_pool.tile([P, 512], F32, tag="out", name="ot")
                        nc.vector.tensor_scalar_mul(ot, ps, recip[:, s:s + 1])
                        nc.sync.dma_start(
                            out=out_d[QB * jb + P * s: QB * jb + P * (s + 1),
                                      512 * n: 512 * (n + 1)],
                            in_=ot)


def build_nc():
    nc = bacc.Bacc("TRN2", target_bir_lowering=False, debug=False, num_devices=8)
    xT_d = nc.dram_tensor("xT", [D, T], BF16, kind="ExternalInput")
    xTk_d = nc.dram_tensor("xTk", [D, T // 2], BF16, kind="ExternalInput")
    xTq_d = nc.dram_tensor("xTq", [D, T // 2], BF16, kind="ExternalInput")
    wq_d = nc.dram_tensor("wq", [D, D], BF16, kind="ExternalInput")
    wk_d = nc.dram_tensor("wk", [D, D], BF16, kind="ExternalInput")
    wv_d = nc.dram_tensor("wv", [D, D], BF16, kind="ExternalInput")
    masks_d = nc.dram_tensor("masks", [4, P, QB], F32, kind="ExternalInput")
    out_d = nc.dram_tensor("out", [T // 2, D], F32, kind="ExternalOutput")
    with tile.TileContext(nc) as tc:
        _emit(nc, tc, xT_d[:], xTk_d[:], xTq_d[:], wq_d[:], wk_d[:], wv_d[:],
              masks_d[:],
              out_d[:])
    nc.compile()
    return nc


def make_masks(h):
    """Additive causal mask: 0 where key (128u + p) <= query (2j + h), else
    -1e9, within a 512-position diagonal window (positions relative to the
    q-block base).  Applied to raw scores before exp."""
    u = np.arange(4)[:, None, None]
    p = np.arange(P)[None, :, None]
    j = np.arange(QB)[None, None, :]
    vis = (128 * u + p <= 2 * j + h)
    return np.where(vis, 0.0, MASK_NEG).astype(np.float32)


def make_in_maps(x, W_query, W_key, W_value):
    wq = np.ascontiguousarray(W_query).astype(BF16_NP)
    wk = np.ascontiguousarray(W_key).astype(BF16_NP)
    wv = np.ascontiguousarray(W_value).astype(BF16_NP)
    masks = [make_masks(h) for h in range(2)]
    in_maps = []
    for core in range(8):
        b, h = divmod(core, 2)
        xb = np.asarray(x[b], dtype=np.float32)
        in_maps.append({
            "xT": np.ascontiguousarray(xb.T).astype(BF16_NP),
            "xTk": np.ascontiguousarray(xb[1024 * h:1024 * (h + 1)].T)
                   .astype(BF16_NP),
            "xTq": np.ascontiguousarray(xb[h::2].T).astype(BF16_NP),
            "wq": wq, "wk": wk, "wv": wv,
            "masks": masks[h],
        })
    return in_maps


_NC_CACHE = {}
LAST_EXEC_NS = None


def kernel(x, W_query, W_key, W_value):
    global LAST_EXEC_NS
    from concourse.bass_utils import run_bass_kernel_spmd

    if "nc" not in _NC_CACHE:
        _NC_CACHE["nc"] = build_nc()
    nc = _NC_CACHE["nc"]

    in_maps = make_in_maps(x, W_query, W_key, W_value)
    trace = bool(os.environ.get("BASS_TRACE"))
    res = run_bass_kernel_spmd(nc, in_maps, core_ids=list(range(8)), trace=trace)
    LAST_EXEC_NS = res.exec_time_ns

    out = np.empty((B, T, D), dtype=np.float32)
    for core in range(8):
        b, h = divmod(core, 2)
        out[b, h::2, :] = res.results[core]["out"]
    return out


if __name__ == "__main__":
    import time
    t0 = time.time()
    nc = build_nc()
    print(f"build+compile took {time.time() - t0:.1f}s")
    n_inst = sum(len(getattr(e, 'instructions', [])) for e in nc.engines) \
        if hasattr(nc, 'engines') else -1
    print("built ok")


# revision 16
# speedup vs baseline: 1.5005x; 1.0009x over previous
"""Causal single-head attention on 8 Trainium2 NeuronCores.

Problem: x[4, 2048, 1024] fp32, Wq/Wk/Wv[1024, 1024] fp32.
  q,k,v = x@Wq, x@Wk, x@Wv ; out = softmax(mask(q k^T)/32) @ v

Sharding (SPMD — one program, 8 cores, per-core data):
  core = 2*b + h  handles batch b, queries {t : t % 2 == h} (1024 queries).
  The interleaved (mod-2) query split makes the causal block structure
  shape-identical across cores: per-core q-block jb (256 queries, spanning
  global positions [512*jb, 512*jb+512)) needs k-tiles 0..4*(jb+1)-1 on
  every core.  Causal masking inside the 4 diagonal k-tiles depends only on
  (u = t-4*jb, parity h) => 4 mask tiles passed as per-core data.

Layouts:
  - host passes x[b]^T ([D, T]) so Q^T/K^T/V all come out of matmuls with
    no on-chip transposes (contraction over d_in needs d_in on partitions).
  - scores are computed transposed ([k, q]) so that expS can feed the
    attn @ V matmul directly as the stationary operand (lhsT = expS[k, q]).
  - softmax denominator: DVE accumulates partition-partial sums, one tiny
    fp32 ones-matmul per q-sub reduces across partitions.  No
    max-subtraction (logits/32 are ~N(0, 0.41^2); exp never overflows).
  - K^T projection is split across the core pair and exchanged with two
    pipelined pair-AllGathers (~1 MB each) that hide behind the V + Q
    projections.  V's gather would be 8 MB / ~109 us — not worth it.

Dtypes: bf16 matmul inputs for projections and scores (fp32 PSUM accum),
float32r V / expS context matmuls (full fp32 rate at N>=256), fp32 softmax.

Measured on HW: ~224 us exec, rel err 2.8e-3 (dominated by bf16 rounding
of x/W/Q/K; identical to the numpy golden model of the same arithmetic).
"""

import os
import numpy as np
import ml_dtypes

import concourse.mybir as mybir
import concourse.tile as tile
from concourse import bacc

F32 = mybir.dt.float32
F32R = mybir.dt.float32r
BF16 = mybir.dt.bfloat16
BF16_NP = ml_dtypes.bfloat16

B, T, D = 4, 2048, 1024
P = 128
DC = D // P          # 8 contraction chunks
NW = T // 512        # 4 token windows (K/V projection granularity)
NT = T // P          # 16 key tiles
QB = 256             # queries per q-block (per core)
NJB = (T // 2) // QB # 4 q-blocks per core
SCALE = 1.0 / 32.0   # 1/sqrt(D)

# V / expS storage dtype: fp32 + float32r matmuls (accurate, full rate at
# N>=256).  Flip to BF16 if hardware shows fp32r matmuls are slow.
V_F32 = True
V_DT = F32R if V_F32 else BF16   # fp32r: walrus requires producers to round
MASK_NEG = -1.0e9
# Split the K^T projection across the core pair (each core projects its own
# 1024 tokens) and AllGather the halves; the 2 MB bf16 gather (~39 us) hides
# completely behind the full V + Q projections.  V stays locally projected —
# its 8 MB gather measured ~109 us and stalls the PE (tried, reverted).
K_SPLIT = True
PAIRS = [[0, 1], [2, 3], [4, 5], [6, 7]]
_EXP = mybir.ActivationFunctionType.Exp


def _emit(nc, tc, xT_d, xTk_d, xTq_d, wq_d, wk_d, wv_d, masks_d, out_d):
    HT = T // 2  # queries per core

    def mm(out, lhsT, rhs, start, stop, **kw):
        if out.dtype == F32 and lhsT.dtype == F32:
            lhsT = lhsT.bitcast(F32R)
            rhs = rhs.bitcast(F32R)
        nc.tensor.matmul(out, lhsT, rhs, start=start, stop=stop, **kw)

    with (
        tc.sbuf_pool(name="persist", bufs=1) as persist,
        tc.sbuf_pool(name="recipp", bufs=2) as recip_pool,
        tc.sbuf_pool(name="accp", bufs=2) as acc_pool,
        tc.sbuf_pool(name="outp", bufs=4) as out_pool,
        tc.psum_pool(name="p512", bufs=4) as p512,
        tc.psum_pool(name="p256", bufs=3) as p256,
        tc.psum_pool(name="pden", bufs=1) as pden,
    ):
        # ---- persistent SBUF tensors ----
        K_sb = persist.tile([P, DC * T], BF16, tag="K", name="K_sb")
        V_sb = persist.tile([P, NT * D], V_DT, tag="V", name="V_sb")
        Q_sb = persist.tile([P, DC * HT], BF16, tag="Q", name="Q_sb")
        expS = persist.tile([P, NT * QB], V_DT, tag="E", name="expS")
        mask_sb = persist.tile([P, 4 * QB], F32, tag="M", name="mask_sb")
        ones_f32 = persist.tile([P, 1], F32, tag="O32", name="ones_f32")
        nc.vector.memset(ones_f32, 1.0)
        for u in range(4):
            nc.sync.dma_start(out=mask_sb[:, u * QB:(u + 1) * QB], in_=masks_d[u])

        # ---- projections: K^T (pair-split + AllGather) and V (local) ----
        with (
            tc.sbuf_pool(name="wkvp", bufs=1) as wkv_pool,
            tc.sbuf_pool(name="xtwp", bufs=2) as xtw_pool,
            tc.sbuf_pool(name="stgp", bufs=6) as stg_pool,
            tc.tile_pool(name="drp", bufs=1, space="DRAM") as dr_pool,
            nc.named_scope("kv_proj"),
        ):
            wk_sb = wkv_pool.tile([P, DC * D], BF16, tag="wk", name="wk_sb")
            wv_sb = wkv_pool.tile([P, DC * D], BF16, tag="wv", name="wv_sb")
            for c in range(DC):
                nc.sync.dma_start(out=wk_sb[:, c * D:(c + 1) * D],
                                  in_=wk_d[c * P:(c + 1) * P, :])
            if K_SPLIT:
                # K^T of own token half first; one pipelined AllGather per
                # 512-token window so gather #0 launches while window 1 is
                # still projecting (pair gathers have ~20us launch latency).
                klocs, kgs = [], []
                for w in range(NW // 2):
                    klocs.append(dr_pool.tile([D, 512], BF16, tag=f"kloc{w}",
                                              name=f"kloc{w}"))
                    kgs.append(dr_pool.tile([2, D, 512], BF16, tag=f"kg{w}",
                                            name=f"kg{w}"))
                for w in range(NW // 2):
                    xtk = xtw_pool.tile([P, DC * 512], BF16, tag="xtw",
                                        name="xtk")
                    for c in range(DC):
                        nc.sync.dma_start(
                            out=xtk[:, c * 512:(c + 1) * 512],
                            in_=xTk_d[c * P:(c + 1) * P, 512 * w:512 * (w + 1)])
                    for c2 in range(DC):
                        ps = p512.tile([P, 512], F32, tag="mm512", name="ps_k")
                        for c in range(DC):
                            mm(ps, wk_sb[:, c * D + P * c2: c * D + P * (c2 + 1)],
                               xtk[:, c * 512:(c + 1) * 512], c == 0, c == DC - 1)
                        st = stg_pool.tile([P, 512], BF16, tag="stk", name="stk")
                        nc.scalar.copy(out=st, in_=ps)
                        nc.sync.dma_start(
                            out=klocs[w][c2 * P:(c2 + 1) * P, :], in_=st)
                    nc.gpsimd.collective_compute(
                        "AllGather", mybir.AluOpType.bypass,
                        replica_groups=PAIRS, ins=[klocs[w][:]],
                        outs=[kgs[w][:]])
                for lw in range(NW // 2):
                    for r in range(2):
                        gw = 2 * r + lw  # global token window
                        for c in range(DC):
                            nc.sync.dma_start(
                                out=K_sb[:, c * T + 512 * gw:
                                         c * T + 512 * (gw + 1)],
                                in_=kgs[lw][r, c * P:(c + 1) * P, :])
            # V (full, local) — PE work here hides the K gather
            for c in range(DC):
                nc.sync.dma_start(out=wv_sb[:, c * D:(c + 1) * D],
                                  in_=wv_d[c * P:(c + 1) * P, :])
            for w in range(NW):
                xtw = xtw_pool.tile([P, DC * 512], BF16, tag="xtw", name="xtw")
                for c in range(DC):
                    nc.sync.dma_start(
                        out=xtw[:, c * 512:(c + 1) * 512],
                        in_=xT_d[c * P:(c + 1) * P, 512 * w:512 * (w + 1)])
                if not K_SPLIT:
                    for c2 in range(DC):
                        ps = p512.tile([P, 512], F32, tag="mm512", name="ps_k")
                        for c in range(DC):
                            mm(ps, wk_sb[:, c * D + P * c2: c * D + P * (c2 + 1)],
                               xtw[:, c * 512:(c + 1) * 512], c == 0, c == DC - 1)
                        nc.scalar.copy(
                            out=K_sb[:, c2 * T + 512 * w: c2 * T + 512 * (w + 1)],
                            in_=ps)
                for ts in range(4):
                    t = 4 * w + ts
                    for n in range(2):
                        ps = p512.tile([P, 512], F32, tag="mm512", name="ps_v")
                        for c in range(DC):
                            mm(ps, xtw[:, c * 512 + P * ts: c * 512 + P * (ts + 1)],
                               wv_sb[:, c * D + 512 * n: c * D + 512 * (n + 1)],
                               c == 0, c == DC - 1)
                        nc.scalar.copy(
                            out=V_sb[:, t * D + 512 * n: t * D + 512 * (n + 1)],
                            in_=ps)

        # ---- Q^T projection (own queries only), per q-block ----
        with (
            tc.sbuf_pool(name="wqp", bufs=1) as wq_pool,
            tc.sbuf_pool(name="xtqp", bufs=2) as xtq_pool,
            nc.named_scope("q_proj"),
        ):
            wq_sb = wq_pool.tile([P, DC * D], BF16, tag="wq", name="wq_sb")
            for c in range(DC):
                nc.sync.dma_start(out=wq_sb[:, c * D:(c + 1) * D],
                                  in_=wq_d[c * P:(c + 1) * P, :])
            for jp in range(NJB // 2):  # project two q-blocks at once (N=512)
                xtq = xtq_pool.tile([P, DC * 512], BF16, tag="xtq", name="xtq")
                for c in range(DC):
                    nc.sync.dma_start(
                        out=xtq[:, c * 512:(c + 1) * 512],
                        in_=xTq_d[c * P:(c + 1) * P, 512 * jp:512 * (jp + 1)])
                for c2 in range(DC):
                    ps = p512.tile([P, 512], F32, tag="mm512", name="ps_q")
                    for c in range(DC):
                        mm(ps, wq_sb[:, c * D + P * c2: c * D + P * (c2 + 1)],
                           xtq[:, c * 512:(c + 1) * 512], c == 0, c == DC - 1)
                    nc.scalar.copy(
                        out=Q_sb[:, c2 * HT + 512 * jp: c2 * HT + 512 * (jp + 1)],
                        in_=ps)

        # ---- attention, per q-block ----
        with nc.named_scope("attn"):
            for jb in range(NJB):
                kt = 4 * (jb + 1)  # k-tiles needed by this q-block
                # pass 1: scores^T -> exp (-> mask on the 4 diagonal tiles)
                for t in range(kt):
                    ps = p256.tile([P, QB], F32, tag="mm256", name="ps_s")
                    for c in range(DC):
                        mm(ps, K_sb[:, c * T + P * t: c * T + P * (t + 1)],
                           Q_sb[:, c * HT + QB * jb: c * HT + QB * (jb + 1)],
                           c == 0, c == DC - 1)
                    if t >= kt - 4:
                        u = t - (kt - 4)
                        nc.vector.tensor_add(ps, ps,
                                             mask_sb[:, u * QB:(u + 1) * QB])
                    nc.scalar.activation(out=expS[:, t * QB:(t + 1) * QB], in_=ps,
                                         func=_EXP, scale=SCALE)
                # denominators: den[q, s] = sum_k expS[k, q].  Partition-
                # partial sums accumulate on the (idle) DVE; one tiny fp32
                # matmul per q-sub does the final cross-partition reduction
                # (N=1 fp32 matmuls are slow on the PE, ~220ns each).
                acc = acc_pool.tile([P, QB], F32, tag="acc", name="acc")
                nc.vector.tensor_copy(acc, expS[:, 0:QB].bitcast(F32))
                for t in range(1, kt):
                    nc.vector.tensor_add(
                        acc, acc, expS[:, t * QB:(t + 1) * QB].bitcast(F32))
                den = pden.tile([P, 2], F32, tag="den", name="den")
                for s in range(2):
                    nc.tensor.matmul(den[:, s:s + 1],
                                     acc[:, P * s:P * (s + 1)], ones_f32,
                                     start=True, stop=True,
                                     skip_group_check=True)
                recip = recip_pool.tile([P, 2], F32, tag="recip", name="recip")
                nc.vector.reciprocal(recip, den)
                # pass 2: ctx[q, d] = sum_k expS[k, q] * V[k, d], then normalize
                for s in range(2):
                    for n in range(2):
                        ps = p512.tile([P, 512], F32, tag="mm512", name="ps_c")
                        for t in range(kt):
                            mm(ps, expS[:, t * QB + P * s: t * QB + P * (s + 1)],
                               V_sb[:, t * D + 512 * n: t * D + 512 * (n + 1)],
                               t == 0, t == kt - 1)
                        ot = out_pool.tile([P, 512], F32, tag="out", name="ot")
                        nc.vector.tensor_scalar_mul(ot, ps, recip[:, s:s + 1])
                        nc.sync.dma_start(
                            out=out_d[QB * jb + P * s: QB * jb + P * (s + 1),
                                      512 * n: 512 * (n + 1)],
                            in_=ot)


def build_nc():
    nc = bacc.Bacc("TRN2", target_bir_lowering=False, debug=False, num_devices=8)
    xT_d = nc.dram_tensor("xT", [D, T], BF16, kind="ExternalInput")
    xTk_d = nc.dram_tensor("xTk", [D, T // 2], BF16, kind="ExternalInput")
    xTq_d = nc.dram_tensor("xTq", [D, T // 2], BF16, kind="ExternalInput")
    wq_d = nc.dram_tensor("wq", [D, D], BF16, kind="ExternalInput")
    wk_d = nc.dram_tensor("wk", [D, D], BF16, kind="ExternalInput")
    wv_d = nc.dram_tensor("wv", [D, D], BF16, kind="ExternalInput")
    masks_d = nc.dram_tensor("masks", [4, P, QB], F32, kind="ExternalInput")
    out_d = nc.dram_tensor("out", [T // 2, D], F32, kind="ExternalOutput")
    with tile.TileContext(nc) as tc:
        _emit(nc, tc, xT_d[:], xTk_d[:], xTq_d[:], wq_d[:], wk_d[:], wv_d[:],
              masks_d[:],
              out_d[:])
    nc.compile()
    return nc


def make_masks(h):
    """Additive causal mask: 0 where key (128u + p) <= query (2j + h), else
    -1e9, within a 512-position diagonal window (positions relative to the
    q-block base).  Applied to raw scores before exp."""
    u = np.arange(4)[:, None, None]
    p = np.arange(P)[None, :, None]
    j = np.arange(QB)[None, None, :]
    vis = (128 * u + p <= 2 * j + h)
    return np.where(vis, 0.0, MASK_NEG).astype(np.float32)


def make_in_maps(x, W_query, W_key, W_value):
    wq = np.ascontiguousarray(W_query).astype(BF16_NP)
    wk = np.ascontiguousarray(W_key).astype(BF16_NP)
    wv = np.ascontiguousarray(W_value).astype(BF16_NP)
    masks = [make_masks(h) for h in range(2)]
    in_maps = []
    for core in range(8):
        b, h = divmod(core, 2)
        xb = np.asarray(x[b], dtype=np.float32)
        in_maps.append({
            "xT": np.ascontiguousarray(xb.T).astype(BF16_NP),
            "xTk": np.ascontiguousarray(xb[1024 * h:1024 * (h + 1)].T)
                   .astype(BF16_NP),
            "xTq": np.ascontiguousarray(xb[h::2].T).astype(BF16_NP),
            "wq": wq, "wk": wk, "wv": wv,
            "masks": masks[h],
        })
    return in_maps


_NC_CACHE = {}
LAST_EXEC_NS = None


def kernel(x, W_query, W_key, W_value):
    global LAST_EXEC_NS
    from concourse.bass_utils import run_bass_kernel_spmd

    if "nc" not in _NC_CACHE:
        _NC_CACHE["nc"] = build_nc()
    nc = _NC_CACHE["nc"]

    in_maps = make_in_maps(x, W_query, W_key, W_value)
    trace = bool(os.environ.get("BASS_TRACE"))
    res = run_bass_kernel_spmd(nc, in_maps, core_ids=list(range(8)), trace=trace)
    LAST_EXEC_NS = res.exec_time_ns

    out = np.empty((B, T, D), dtype=np.float32)
    for core in range(8):
        b, h = divmod(core, 2)
        out[b, h::2, :] = res.results[core]["out"]
    return out


if __name__ == "__main__":
    import time
    t0 = time.time()
    nc = build_nc()
    print(f"build+compile took {time.time() - t0:.1f}s")
    n_inst = sum(len(getattr(e, 'instructions', [])) for e in nc.engines) \
        if hasattr(nc, 'engines') else -1
    print("built ok")
